# revision 1
# baseline (speedup 1.0000x reference)
"""BBox spatial attention kernel for Trainium2 (8 NeuronCores, data-parallel over B).

Reference math per batch b, box n:
    gauss[n, y, x] = exp(-(dx2[n, x] + dy2[n, y]))
    att[y, x]      = max_n gauss   (all-zero boxes masked out)

exp is monotonic, so att = exp(-min_n (dx2[n,x] + dy2[n,y])). The additive
z[n,y,x] = dy2[n,y] + dx2[n,x] field is rank-2, so each 4-box group of z
tiles is built by ONE K=34 float32r matmul straight into a PSUM bank:
  rows 0..31:  lhsT = DY2'[32, 128] (dy2 + invalid-box penalty), rhs = const
               block-diagonal ones -> routes box m's dy2 column into its own
               128-column block.
  rows 32/33:  lhsT = batch-selector ones pattern, rhs = DXF_b[1, 4096]
               (all boxes' dx2 flattened onto one partition) -> broadcasts
               dx2 across all y partitions.
The min over boxes is a strided reduce_min on the VectorEngine straight out
of PSUM, then one Exp per batch on the ScalarEngine. Invalid (all-zero)
boxes get +1e5 added to dy2 -> exp -> exact 0. feature_map only provides
H/W and is never touched.

Sharding: B=16 -> 2 batches per core, 8 cores, no cross-core comms.
"""

import math

import numpy as np

import concourse.bacc as bacc
import concourse.bass as bass
import concourse.mybir as mybir
import concourse.tile as tile
from concourse.bass_utils import run_bass_kernel_spmd

B, N, H, W = 16, 32, 128, 128
N_CORES = 8
B_LOC = B // N_CORES  # 2 batches per core
EPS = 1e-6
F32 = mybir.dt.float32
F32R = mybir.dt.float32r
AX = mybir.AxisListType
ALU = mybir.AluOpType
ACT = mybir.ActivationFunctionType

_CACHE: dict = {}


def _diag_ones() -> np.ndarray:
    d = np.zeros((N, N * W), dtype=np.float32)
    for r in range(N):
        d[r, r * W : (r + 1) * W] = 1.0
    return d


def _bsel_ones() -> np.ndarray:
    s = np.zeros((B_LOC, B_LOC * W), dtype=np.float32)
    for b in range(B_LOC):
        s[b, b * W : (b + 1) * W] = 1.0
    return s


def build_nc(reps: int = 1):
    nc = bacc.Bacc(
        "TRN2",
        target_bir_lowering=False,
        debug=False,
        enable_asserts=False,
    )
    bb = nc.dram_tensor("bb", [B_LOC, N, 4], F32, kind="ExternalInput")
    att = nc.dram_tensor("att", [B_LOC, H, W], F32, kind="ExternalOutput")
    iota2_dram = nc.inline_tensor(
        np.tile(2.0 * np.arange(W, dtype=np.float32), (N, 1)), name="iota2_const"
    )
    diag_dram = nc.inline_tensor(_diag_ones(), name="diag_const")
    bsel_dram = nc.inline_tensor(_bsel_ones(), name="bsel_const")

    with tile.TileContext(nc) as tc:
        with (
            tc.tile_pool(name="sb", bufs=1) as sb,
            tc.tile_pool(name="psum", bufs=2, space="PSUM") as pp,
        ):
            # tiny warmup activation so the ACT function-table load (~1.3us)
            # happens at t=0 instead of blocking the first real Square
            warm = sb.tile([128, 1], F32, tag="warm")
            nc.vector.memset(warm[:], 0.0)
            nc.scalar.activation(warm[:], warm[:], ACT.Square)

            for _rep in range(reps):
                _body(nc, sb, pp, bb, att, iota2_dram, diag_dram, bsel_dram)

    nc.compile()
    return nc


def _body(nc, sb, pp, bb, att, iota2_dram, diag_dram, bsel_dram):
    # all tiles are 128-partition so every matmul operand sits at base
    # partition 0 (PE tile_position (0, 0))
    bbt = sb.tile([128, B_LOC * 4], F32, tag="bbt")  # [n, (b c)]
    nc.sync.dma_start(
        bbt[0:N, :].rearrange("p (b c) -> p b c", b=B_LOC),
        bb.ap().rearrange("b n c -> n b c"),
    )
    iota2 = sb.tile([128, W], F32, tag="iota2")
    nc.sync.dma_start(iota2[0:N, :], iota2_dram.ap())
    # diag rows 0..31: const block-diagonal ones; rows 32/33: dx2 flat (dyn)
    diag = sb.tile([128, N * W], F32R, tag="diag")
    nc.sync.dma_start(diag[0:N, :], diag_dram.ap().bitcast(F32R))
    # uyp rows 0..31: dy2'; rows 32/33: batch-selector ones pattern (const)
    uyp = sb.tile([128, B_LOC * H], F32R, tag="uyp")
    nc.sync.dma_start(uyp[N : N + B_LOC, :], bsel_dram.ap().bitcast(F32R))

    # --- per-box params, boxes on partitions 0..31, b along free ---
    # pixel coords: clip(floor(v*128), 0, 127); v*128 exact (pow2).
    # floor via round-half magic: a = fl(v + (2^23 - 0.5)) = RNE(v - 0.5)+2^23
    # (exact for v in [0, 2^22) with frac(v) != 0; inputs are uniform [0,1)
    # so v is never an exact integer). b = -max(a, 2^23) clamps negatives,
    # fn = b + 2^23 = -clip(floor(v), 0, inf); upper clip unneeded (v < 128).
    MAGIC = 8388608.0  # 2^23
    a = sb.tile([128, 8], F32, tag="a")
    nc.vector.tensor_scalar(
        a[0:N, :], bbt[0:N, :], float(W), MAGIC - 0.5, ALU.mult, ALU.add
    )
    bm = sb.tile([128, 8], F32, tag="bm")
    nc.vector.tensor_scalar(bm[0:N, :], a[0:N, :], MAGIC, -1.0, ALU.max, ALU.mult)
    # s[:, 2k+b] = hi-lo box extent (from bm directly; the 2^23 offsets cancel)
    bv = bm[0:N, :].rearrange("p (b c) -> p b c", b=B_LOC)
    s = sb.tile([128, 4], F32, tag="s")
    nc.vector.tensor_tensor(
        s[0:N, :].rearrange("p (k b) -> p b k", k=2),
        bv[:, :, 0:2],
        bv[:, :, 2:4],
        ALU.subtract,
    )
    # d = 2*sqrt(2)*(s*0.25 + eps); r2 = 1/d so (2x - c)*r2 = (x-cx)/(sqrt2*sx)
    d = sb.tile([128, 4], F32, tag="d")
    nc.vector.tensor_scalar(
        d[0:N, :],
        s[0:N, :],
        math.sqrt(2.0) / 2.0,
        2.0 * math.sqrt(2.0) * EPS,
        ALU.mult,
        ALU.add,
    )
    r2 = sb.tile([128, 4], F32, tag="r2")
    nc.vector.reciprocal(r2[0:N, :], d[0:N, :])

    # fn = -clip(floor, 0, 127); cn = -(lo+hi) = -c
    fn = sb.tile([128, 8], F32, tag="fn")
    nc.vector.tensor_scalar(fn[0:N, :], bm[0:N, :], MAGIC, None, ALU.add)
    fv = fn[0:N, :].rearrange("p (b c) -> p b c", b=B_LOC)
    cn = sb.tile([128, 4], F32, tag="cn")
    nc.vector.tensor_tensor(
        cn[0:N, :].rearrange("p (k b) -> p b k", k=2),
        fv[:, :, 2:4],
        fv[:, :, 0:2],
        ALU.add,
    )

    # t4 block j = (iota2 + cn_j) * r2_j = (2x - c)/(2*sqrt2*s2); the
    # subtraction happens exactly BEFORE the multiply (avoids catastrophic
    # cancellation for narrow boxes). blocks j = (k, b):
    # [tx b0 | tx b1 | ty b0 | ty b1]. x blocks first -> flatten DMA ASAP.
    t4 = sb.tile([128, 4 * W], F32, tag="t4")
    u4 = sb.tile([128, 4 * W], F32, tag="u4")
    for j in range(4):
        nc.vector.tensor_scalar(
            t4[0:N, j * W : (j + 1) * W],
            iota2[0:N, :],
            cn[0:N, j : j + 1],
            r2[0:N, j : j + 1],
            ALU.add,
            ALU.mult,
        )
        if j == 1:
            nc.scalar.activation(
                u4[0:N, 0 : 2 * W], t4[0:N, 0 : 2 * W], ACT.Square
            )
            for jj in range(2):
                nc.sync.dma_start(
                    diag[N + jj : N + jj + 1, :],
                    u4[0:N, jj * W : (jj + 1) * W].bitcast(F32R),
                )
    nc.scalar.activation(u4[0:N, 2 * W : 4 * W], t4[0:N, 2 * W : 4 * W], ACT.Square)

    # all-zero-box mask -> +1e5 penalty added to dy2 (runs during ACT work)
    s4 = sb.tile([128, 2], F32, tag="s4")
    nc.vector.reduce_sum(
        s4[0:N, :], bbt[0:N, :].rearrange("p (b c) -> p b c", b=B_LOC), axis=AX.X
    )
    pen = sb.tile([128, 2], F32, tag="pen")
    nc.vector.tensor_scalar(
        pen[0:N, :], s4[0:N, :], 0.0, 1.0e5, ALU.is_equal, ALU.mult
    )
    for b in range(B_LOC):
        nc.vector.tensor_scalar(
            uyp[0:N, b * H : (b + 1) * H],
            u4[0:N, (2 + b) * H : (3 + b) * H],
            pen[0:N, b : b + 1],
            None,
            ALU.add,
        )

    # z = dy2' + dx2 in PSUM via one K=34 f32r matmul per 4-box group;
    # strided reduce_min on DVE straight out of PSUM. Chunked (1, 3, 4)
    # groups per batch so the first reduce starts after a single matmul;
    # chunk slots (1+3+4 banks = full PSUM) ping-pong between batches.
    K = N + B_LOC  # 34
    CHUNKS = (1, 3, 4)
    for b in range(B_LOC):
        mns = []
        gbase = 0
        for nch, ngrp in enumerate(CHUNKS):
            pt = pp.tile([H, ngrp * 512], F32, tag=f"pt{nch}", bufs=1)
            for gl in range(ngrp):
                nc.tensor.matmul(
                    pt[:, 512 * gl : 512 * (gl + 1)],
                    uyp[0:K, b * H : (b + 1) * H],
                    diag[0:K, 512 * (gbase + gl) : 512 * (gbase + gl + 1)],
                    start=True,
                    stop=True,
                )
            gbase += ngrp
            mn = sb.tile([H, W], F32, tag=f"mn{nch}")
            nc.vector.tensor_reduce(
                mn[:],
                pt[:].rearrange("p (i x) -> p x i", i=4 * ngrp),
                axis=AX.X,
                op=ALU.min,
            )
            mns.append(mn)
        nma = sb.tile([H, W], F32, tag="nma")
        nc.vector.tensor_tensor(nma[:], mns[0][:], mns[1][:], ALU.min)
        nmb = sb.tile([H, W], F32, tag="nmb")
        nc.vector.tensor_tensor(nmb[:], nma[:], mns[2][:], ALU.min)
        res = sb.tile([H, W], F32, tag="res")
        nc.scalar.activation(res[:], nmb[:], ACT.Exp, scale=-1.0)
        nc.sync.dma_start(att.ap()[b], res[:])


def _get_nc():
    if "nc" not in _CACHE:
        _CACHE["nc"] = build_nc()
    return _CACHE["nc"]


def kernel(feature_map: np.ndarray, bboxes: np.ndarray) -> np.ndarray:
    nc = _get_nc()
    bb = np.ascontiguousarray(bboxes, dtype=np.float32)
    in_maps = [
        {"bb": bb[c * B_LOC : (c + 1) * B_LOC]} for c in range(N_CORES)
    ]
    res = run_bass_kernel_spmd(nc, in_maps, list(range(N_CORES)))
    out = np.concatenate([res.results[c]["att"] for c in range(N_CORES)], axis=0)
    return out[:, None, :, :].astype(np.float32, copy=False)



# revision 10
# speedup vs baseline: 1.9655x; 1.9655x over previous
"""BBox spatial attention kernel for Trainium2 (8 NeuronCores, data-parallel over B).

Reference math per batch b, box n:
    gauss[n, y, x] = exp(-(dy2[n, y] + dx2[n, x]))
    att[y, x]      = max_n gauss      (all-zero boxes masked out)

Because each gauss plane is rank-1 separable, the p-norm power trick turns the
max into ONE K=32 matmul per batch per power level:
    S_p[y, x] = sum_n (2^29 g^p)[n, y] * (2^29 g^p)[n, x],   (sum g^p)^(1/p) -> max g
(2^29 per factor keeps S <= 2^63, inside the Scalar-Engine Ln window: measured
on HW, Ln is exact on [2^-64, 2^64], saturates to -45.8614 below 2^-64, and
returns garbage above 2^64.)
Levels p = 28, 56 are combined with a 2-term Richardson correction
(exact for two-way ties):  m^112 = S56 * (1 + sqrt(1 - (S28^2/S56 - 1)^2)) / 2,
evaluated in log space with a quadratic fit of
g(D) = ln((1+sqrt(1-(e^D-1)^2))/2) on D in [0, ln2].  A min-cascade over the
level estimates (p-norm monotonicity: each level over-estimates, deeper
levels are tighter but underflow earlier) picks the deepest alive level per
pixel with a plain elementwise min - no selects:
    out = exp(min(2*L28 - 40.20, L56 + g(D)) / 56 - 40.20/56)
plus a third shallow level p = 14 (cand = 4*L14 - 3*40.20) covering the far
field.  The HW Ln saturation value -45.8614 acts as a free dead-level floor:
a level whose S drops below 2^-64 reports exp(-86.06/p) - exactly its
window-cliff bound (>= truth there), so the min discards dead levels and the
p=14 floor exp(-6.15) = 0.0021 bounds the far-field error.
Validated vs the fp64 reference on the seed-0 inputs: rel err 1.14e-2 (tol 2e-2).

Layout: partitions = (b n) [64 rows], free = [y-block 0:128 | x-block 128:256].
All-zero boxes get +1e5 folded into the Exp bias vector -> factors exactly 0.

Sharding: B=16 -> 2 batches per core, 8 cores, no cross-core comms.
feature_map only provides H/W and is never touched.
"""

import math

import numpy as np

import concourse.bacc as bacc
import concourse.bass as bass
import concourse.mybir as mybir
import concourse.tile as tile
from concourse.bass_utils import run_bass_kernel_spmd

B, N, H, W = 16, 32, 128, 128
N_CORES = 8
B_LOC = B // N_CORES  # 2 batches per core
EPS = 1e-6
F32 = mybir.dt.float32
F16 = mybir.dt.float16
BF16 = mybir.dt.bfloat16
ALU = mybir.AluOpType
ACT = mybir.ActivationFunctionType
AX = mybir.AxisListType

MAGIC = 8388608.0  # 2^23
LN2 = math.log(2.0)
CL = 29 * LN2  # factor scale 2^29
K58 = 58 * LN2  # product scale ln(2^58)
LNB = 1e-30  # uniform Ln bias: avoids ln(0) = -inf; HW floors it to -45.8614
PC1 = 0.967875  # g(D) ~ PC2*D^2 + PC1*D, pinned at g(ln2) = -ln2
PC2 = -2.839043

_CACHE: dict = {}


def build_nc(reps: int = 1):
    nc = bacc.Bacc(
        "TRN2",
        target_bir_lowering=False,
        debug=False,
        enable_asserts=False,
    )
    bb = nc.dram_tensor("bb", [B_LOC, N, 4], F32, kind="ExternalInput")
    att = nc.dram_tensor("att", [B_LOC, H, W], F32, kind="ExternalOutput")
    iota2_dram = nc.inline_tensor(
        np.tile(2.0 * np.arange(W, dtype=np.float32), (2 * N, 1)), name="iota2_const"
    )

    with tile.TileContext(nc) as tc:
        with (
            tc.tile_pool(name="sb", bufs=2) as sb,
            tc.tile_pool(name="cst", bufs=1) as cst,
            tc.tile_pool(name="psum", bufs=2, space="PSUM") as pp,
        ):
            # one-time: ACT table (exp+ln set) warm at t=0, const loads
            warm = cst.tile([128, 1], F32, tag="warm")
            nc.vector.memset(warm[:], 1.0)
            nc.scalar.activation(warm[:], warm[:], ACT.Exp)
            nc.scalar.activation(warm[:], warm[:], ACT.Ln)
            iota2 = cst.tile([2 * N, W], F32, tag="iota2")
            nc.sync.dma_start(iota2[:], iota2_dram.ap())
            b0c = cst.tile([128, 1], F32, tag="b0c")
            nc.vector.memset(b0c[:], LNB)
            ebc = cst.tile([128, 1], F32, tag="ebc")
            nc.vector.memset(ebc[:], -K58 / 56.0)

            for _rep in range(reps):
                _body(nc, sb, pp, bb, att, iota2, b0c, ebc)

    nc.compile()
    return nc


def _body(nc, sb, pp, bb, att, iota2, b0c, ebc):
    # bbt[(b n), c]: c = (x1, y1, x2, y2)
    bbt = sb.tile([2 * N, 4], F32, tag="bbt")
    nc.sync.dma_start(bbt[:], bb.ap().rearrange("b n c -> (b n) c"))

    # --- per-box params on Pool; columns (x, y) ---
    # pixel coords clip(floor(v*128), 0, 127) via round-half magic:
    # a = v*128 + (2^23 - 0.5) rounds RNE to 2^23 + floor (v in [0,1)).
    a = sb.tile([2 * N, 4], F32, tag="a")
    nc.gpsimd.tensor_scalar(a[:], bbt[:], float(W), MAGIC - 0.5, ALU.mult, ALU.add)
    bm = sb.tile([2 * N, 4], F32, tag="bm")
    nc.gpsimd.tensor_scalar(bm[:], a[:], MAGIC, -1.0, ALU.max, ALU.mult)
    # fn = -clip(floor), small magnitude (the 2^23 offsets cancel EXACTLY;
    # summing bm directly would round at the 2^24 boundary)
    fn = sb.tile([2 * N, 4], F32, tag="fn")
    nc.gpsimd.tensor_scalar(fn[:], bm[:], MAGIC, None, ALU.add)
    s = sb.tile([2 * N, 2], F32, tag="s")
    nc.gpsimd.tensor_tensor(s[:], bm[:, 0:2], bm[:, 2:4], ALU.subtract)  # hi-lo
    d = sb.tile([2 * N, 2], F32, tag="d")
    nc.gpsimd.tensor_scalar(
        d[:], s[:], math.sqrt(2.0) / 2.0, 2.0 * math.sqrt(2.0) * EPS,
        ALU.mult, ALU.add,
    )
    c0 = sb.tile([2 * N, 2], F32, tag="c0")
    nc.gpsimd.tensor_tensor(c0[:], fn[:, 0:2], fn[:, 2:4], ALU.add)  # -(lo+hi)
    r2 = sb.tile([2 * N, 2], F32, tag="r2")
    nc.vector.reciprocal(r2[:], d[:])
    # all-zero box -> +1e5 on u via the exp bias vectors
    sp_ = sb.tile([2 * N, 2], F32, tag="sp_")
    nc.gpsimd.tensor_tensor(sp_[:], bbt[:, 0:2], bbt[:, 2:4], ALU.add)
    s4 = sb.tile([2 * N, 1], F32, tag="s4")
    nc.gpsimd.tensor_tensor(s4[:], sp_[:, 0:1], sp_[:, 1:2], ALU.add)
    pz = sb.tile([2 * N, 1], F32, tag="pz")
    nc.gpsimd.tensor_scalar(pz[:], s4[:], 0.0, None, ALU.is_equal)
    bv0 = sb.tile([2 * N, 1], F32, tag="bv0")
    nc.gpsimd.tensor_scalar(bv0[:], pz[:], -1.4e6, CL, ALU.mult, ALU.add)
    bv1 = sb.tile([2 * N, 1], F32, tag="bv1")
    nc.gpsimd.tensor_scalar(bv1[:], bv0[:], 2.0, -CL, ALU.mult, ALU.add)
    bv2 = sb.tile([2 * N, 1], F32, tag="bv2")
    nc.gpsimd.tensor_scalar(bv2[:], bv1[:], 2.0, -CL, ALU.mult, ALU.add)

    # t = (2j - lo - hi) / (2*sqrt2*(0.25*(hi-lo) + eps));  u = t^2 = d?2
    t = sb.tile([2 * N, 2 * W], F32, tag="t")
    nc.vector.tensor_scalar(
        t[:, 0:W], iota2[:], c0[:, 1:2], r2[:, 1:2], ALU.add, ALU.mult
    )
    nc.vector.tensor_scalar(
        t[:, W : 2 * W], iota2[:], c0[:, 0:1], r2[:, 0:1], ALU.add, ALU.mult
    )
    u = sb.tile([2 * N, 2 * W], F32, tag="u")
    nc.gpsimd.tensor_tensor(u[:], t[:], t[:], ALU.mult)

    # factors 2^29 * g^p, bf16 (quantization shrinks by the 1/112 root)
    e1 = sb.tile([2 * N, 2 * W], BF16, tag="e1")
    nc.scalar.activation(e1[:], u[:], ACT.Exp, bias=bv0[:], scale=-14.0)
    e2 = sb.tile([2 * N, 2 * W], BF16, tag="e2")
    nc.scalar.activation(e2[:], u[:], ACT.Exp, bias=bv1[:], scale=-28.0)
    e4 = sb.tile([2 * N, 2 * W], BF16, tag="e4")
    nc.scalar.activation(e4[:], u[:], ACT.Exp, bias=bv2[:], scale=-56.0)

    # one PSUM bank (512 f32) per matmul group: HW rejects two accumulation
    # groups in one bank (CoreSim does not model this). 6 banks, single-buffered.
    BK = 512
    ps = pp.tile([128, 6 * BK], F32, tag="ps", bufs=1)
    for lv, e in enumerate((e1, e2, e4)):
        for b in range(B_LOC):
            nc.tensor.matmul(
                ps[:, (2 * lv + b) * BK : (2 * lv + b) * BK + W],
                e[32 * b : 32 * (b + 1), 0:W],          # y-factors (lhsT)
                e[32 * b : 32 * (b + 1), W : 2 * W],    # x-factors (rhs)
                start=True,
                stop=True,
            )

    # L = fp16(ln(S + 1e-30)); [L14 | L28 | L56] blocks of 256
    L = sb.tile([128, 6 * W], F16, tag="L")
    psv = ps[:].rearrange("p (q c) -> p q c", q=6)[:, :, 0:W]
    nc.scalar.activation(L[:], psv, ACT.Ln, bias=b0c[:], scale=1.0)
    L1 = L[:, 0 : 2 * W]
    L2 = L[:, 2 * W : 4 * W]
    L4 = L[:, 4 * W : 6 * W]

    # min-cascade in 64*log domain (all fp16, 2x/4x DVE modes)
    a2 = sb.tile([128, 2 * W], F16, tag="a2")
    nc.vector.tensor_scalar(a2[:], L2, 2.0, -K58, ALU.mult, ALU.add)
    dd = sb.tile([128, 2 * W], F16, tag="dd")
    nc.vector.tensor_tensor(dd[:], a2[:], L4, ALU.subtract)
    dc = sb.tile([128, 2 * W], F16, tag="dc")
    nc.vector.tensor_scalar(dc[:], dd[:], 0.0, LN2, ALU.max, ALU.min)
    h = sb.tile([128, 2 * W], F16, tag="h")
    nc.vector.tensor_scalar(h[:], dc[:], PC2, PC1, ALU.mult, ALU.add)
    g = sb.tile([128, 2 * W], F16, tag="g")
    nc.vector.tensor_tensor(g[:], h[:], dc[:], ALU.mult)
    a4 = sb.tile([128, 2 * W], F16, tag="a4")
    nc.vector.tensor_tensor(a4[:], L4, g[:], ALU.add)
    c14 = sb.tile([128, 2 * W], F16, tag="c14")
    nc.vector.tensor_scalar(c14[:], L1, 4.0, -3.0 * K58, ALU.mult, ALU.add)
    m1 = sb.tile([128, 2 * W], F16, tag="m1")
    nc.vector.tensor_tensor(m1[:], a4[:], a2[:], ALU.min)
    marg = sb.tile([128, 2 * W], F16, tag="marg")
    nc.vector.tensor_tensor(marg[:], m1[:], c14[:], ALU.min)

    res = sb.tile([128, 2 * W], F32, tag="res")
    nc.scalar.activation(
        res[:], marg[:], ACT.Exp, bias=ebc[:], scale=1.0 / 56.0
    )
    nc.sync.dma_start(
        att.ap().rearrange("b y x -> y b x"),
        res[:].rearrange("p (b x) -> p b x", b=B_LOC),
    )


def _get_nc():
    if "nc" not in _CACHE:
        _CACHE["nc"] = build_nc()
    return _CACHE["nc"]


def kernel(feature_map: np.ndarray, bboxes: np.ndarray) -> np.ndarray:
    nc = _get_nc()
    bb = np.ascontiguousarray(bboxes, dtype=np.float32)
    in_maps = [
        {"bb": bb[c * B_LOC : (c + 1) * B_LOC]} for c in range(N_CORES)
    ]
    res = run_bass_kernel_spmd(nc, in_maps, list(range(N_CORES)))
    out = np.concatenate([res.results[c]["att"] for c in range(N_CORES)], axis=0)
    return out[:, None, :, :].astype(np.float32, copy=False)


# revision 11
# speedup vs baseline: 2.6867x; 1.3670x over previous
"""BBox spatial attention kernel for Trainium2 (8 NeuronCores, data-parallel over B).

Reference math per batch b, box n:
    gauss[n, y, x] = exp(-(dy2[n, y] + dx2[n, x]))
    att[y, x]      = max_n gauss      (all-zero boxes masked out)

Because each gauss plane is rank-1 separable, the p-norm power trick turns the
max into ONE K=32 matmul per batch per power level:
    S_p[y, x] = sum_n (2^29 g^p)[n, y] * (2^29 g^p)[n, x],   (sum g^p)^(1/p) -> max g
(2^29 per factor keeps S <= 2^63, inside the Scalar-Engine Ln window: measured
on HW, Ln is exact on [2^-64, 2^64], saturates to -45.8614 below 2^-64, and
returns garbage above 2^64.)
Levels p = 28, 56 are combined with a 2-term Richardson correction
(exact for two-way ties):  m^112 = S56 * (1 + sqrt(1 - (S28^2/S56 - 1)^2)) / 2,
evaluated in log space with a quadratic fit of
g(D) = ln((1+sqrt(1-(e^D-1)^2))/2) on D in [0, ln2].  A min-cascade over the
level estimates (p-norm monotonicity: each level over-estimates, deeper
levels are tighter but underflow earlier) picks the deepest alive level per
pixel with a plain elementwise min - no selects:
    out = exp(min(2*L28 - 40.20, L56 + g(D)) / 56 - 40.20/56)
plus a third shallow level p = 14 (cand = 4*L14 - 3*40.20) covering the far
field.  The HW Ln saturation value -45.8614 acts as a free dead-level floor:
a level whose S drops below 2^-64 reports exp(-86.06/p) - exactly its
window-cliff bound (>= truth there), so the min discards dead levels and the
p=14 floor exp(-6.15) = 0.0021 bounds the far-field error.
Validated vs the fp64 reference on the seed-0 inputs: rel err 1.14e-2 (tol 2e-2).

Layout: partitions = (b n) [64 rows], free = [y-block 0:128 | x-block 128:256].
All-zero boxes get +1e5 folded into the Exp bias vector -> factors exactly 0.

Sharding: B=16 -> 2 batches per core, 8 cores, no cross-core comms.
feature_map only provides H/W and is never touched.
"""

import math

import numpy as np

import concourse.bacc as bacc
import concourse.bass as bass
import concourse.mybir as mybir
import concourse.tile as tile
from concourse.bass_utils import run_bass_kernel_spmd

B, N, H, W = 16, 32, 128, 128
N_CORES = 8
B_LOC = B // N_CORES  # 2 batches per core
EPS = 1e-6
F32 = mybir.dt.float32
F16 = mybir.dt.float16
BF16 = mybir.dt.bfloat16
ALU = mybir.AluOpType
ACT = mybir.ActivationFunctionType
AX = mybir.AxisListType

MAGIC = 8388608.0  # 2^23
LN2 = math.log(2.0)
CL = 29 * LN2  # factor scale 2^29
K58 = 58 * LN2  # product scale ln(2^58)
LNB = 1e-30  # uniform Ln bias: avoids ln(0) = -inf; HW floors it to -45.8614
PC1 = 0.967875  # g(D) ~ PC2*D^2 + PC1*D, pinned at g(ln2) = -ln2
PC2 = -2.839043

_CACHE: dict = {}


def build_nc(reps: int = 1):
    nc = bacc.Bacc(
        "TRN2",
        target_bir_lowering=False,
        debug=False,
        enable_asserts=False,
    )
    bb = nc.dram_tensor("bb", [B_LOC, N, 4], F32, kind="ExternalInput")
    att = nc.dram_tensor("att", [B_LOC, H, W], F32, kind="ExternalOutput")
    iota2_dram = nc.inline_tensor(
        np.tile(2.0 * np.arange(W, dtype=np.float32), (2 * N, 1)), name="iota2_const"
    )

    with tile.TileContext(nc) as tc:
        with (
            tc.tile_pool(name="sb", bufs=2) as sb,
            tc.tile_pool(name="cst", bufs=1) as cst,
            tc.tile_pool(name="psum", bufs=2, space="PSUM") as pp,
        ):
            # one-time: ACT table (exp+ln set) warm at t=0, const loads
            warm = cst.tile([128, 1], F32, tag="warm")
            nc.vector.memset(warm[:], 1.0)
            nc.scalar.activation(warm[:], warm[:], ACT.Exp)
            nc.scalar.activation(warm[:], warm[:], ACT.Ln)
            iota2 = cst.tile([2 * N, W], F32, tag="iota2")
            nc.sync.dma_start(iota2[:], iota2_dram.ap())
            b0c = cst.tile([128, 1], F32, tag="b0c")
            nc.vector.memset(b0c[:], LNB)
            ebc = cst.tile([128, 1], F32, tag="ebc")
            nc.vector.memset(ebc[:], -K58 / 56.0)

            for _rep in range(reps):
                _body(nc, sb, pp, bb, att, iota2, b0c, ebc)

    nc.compile()
    _collapse_act_table_loads(nc)
    return nc


def _collapse_act_table_loads(nc):
    """All activation funcs used here (Exp, Ln) live together in the
    natural_log_exp_and_others table, but the greedy insertion pass picks
    exp_and_others/natural_log alternately, reloading the table RAM twice
    per rep (~2.6 us).  Keep one load, pointed at the combined set."""
    from concourse.hw_specs import get_activation_tables

    names = list(get_activation_tables(nc.m.arch))
    combined = names.index("natural_log_exp_and_others")
    first = True
    for blk in nc.m.functions[0].blocks:
        keep = []
        for inst in blk.instructions:
            if isinstance(inst, mybir.InstLoadActFuncSet):
                if not first:
                    continue
                inst.act_func_set_id = combined
                first = False
            keep.append(inst)
        if len(keep) != len(blk.instructions):
            blk.instructions[:] = keep


def _body(nc, sb, pp, bb, att, iota2, b0c, ebc):
    # bbt[(b n), c]: c = (x1, y1, x2, y2)
    bbt = sb.tile([2 * N, 4], F32, tag="bbt")
    nc.sync.dma_start(bbt[:], bb.ap().rearrange("b n c -> (b n) c"))

    # --- per-box params on Pool; columns (x, y) ---
    # pixel coords clip(floor(v*128), 0, 127) via round-half magic:
    # a = v*128 + (2^23 - 0.5) rounds RNE to 2^23 + floor (v in [0,1)).
    a = sb.tile([2 * N, 4], F32, tag="a")
    nc.gpsimd.tensor_scalar(a[:], bbt[:], float(W), MAGIC - 0.5, ALU.mult, ALU.add)
    bm = sb.tile([2 * N, 4], F32, tag="bm")
    nc.gpsimd.tensor_scalar(bm[:], a[:], MAGIC, -1.0, ALU.max, ALU.mult)
    # fn = -clip(floor), small magnitude (the 2^23 offsets cancel EXACTLY;
    # summing bm directly would round at the 2^24 boundary)
    fn = sb.tile([2 * N, 4], F32, tag="fn")
    nc.gpsimd.tensor_scalar(fn[:], bm[:], MAGIC, None, ALU.add)
    s = sb.tile([2 * N, 2], F32, tag="s")
    nc.gpsimd.tensor_tensor(s[:], bm[:, 0:2], bm[:, 2:4], ALU.subtract)  # hi-lo
    d = sb.tile([2 * N, 2], F32, tag="d")
    nc.gpsimd.tensor_scalar(
        d[:], s[:], math.sqrt(2.0) / 2.0, 2.0 * math.sqrt(2.0) * EPS,
        ALU.mult, ALU.add,
    )
    c0 = sb.tile([2 * N, 2], F32, tag="c0")
    nc.gpsimd.tensor_tensor(c0[:], fn[:, 0:2], fn[:, 2:4], ALU.add)  # -(lo+hi)
    r2 = sb.tile([2 * N, 2], F32, tag="r2")
    nc.vector.reciprocal(r2[:], d[:])
    # all-zero box -> +1e5 on u via the exp bias vectors
    sp_ = sb.tile([2 * N, 2], F32, tag="sp_")
    nc.gpsimd.tensor_tensor(sp_[:], bbt[:, 0:2], bbt[:, 2:4], ALU.add)
    s4 = sb.tile([2 * N, 1], F32, tag="s4")
    nc.gpsimd.tensor_tensor(s4[:], sp_[:, 0:1], sp_[:, 1:2], ALU.add)
    pz = sb.tile([2 * N, 1], F32, tag="pz")
    nc.gpsimd.tensor_scalar(pz[:], s4[:], 0.0, None, ALU.is_equal)
    bv0 = sb.tile([2 * N, 1], F32, tag="bv0")
    nc.gpsimd.tensor_scalar(bv0[:], pz[:], -1.4e6, CL, ALU.mult, ALU.add)
    bv1 = sb.tile([2 * N, 1], F32, tag="bv1")
    nc.gpsimd.tensor_scalar(bv1[:], bv0[:], 2.0, -CL, ALU.mult, ALU.add)
    bv2 = sb.tile([2 * N, 1], F32, tag="bv2")
    nc.gpsimd.tensor_scalar(bv2[:], bv1[:], 2.0, -CL, ALU.mult, ALU.add)

    # t = (2j - lo - hi) / (2*sqrt2*(0.25*(hi-lo) + eps));  u = t^2 = d?2
    t = sb.tile([2 * N, 2 * W], F32, tag="t")
    nc.vector.tensor_scalar(
        t[:, 0:W], iota2[:], c0[:, 1:2], r2[:, 1:2], ALU.add, ALU.mult
    )
    nc.vector.tensor_scalar(
        t[:, W : 2 * W], iota2[:], c0[:, 0:1], r2[:, 0:1], ALU.add, ALU.mult
    )
    u = sb.tile([2 * N, 2 * W], F32, tag="u")
    nc.gpsimd.tensor_tensor(u[:], t[:], t[:], ALU.mult)

    # factors 2^29 * g^p, bf16 (quantization shrinks by the 1/112 root)
    e1 = sb.tile([2 * N, 2 * W], BF16, tag="e1")
    nc.scalar.activation(e1[:], u[:], ACT.Exp, bias=bv0[:], scale=-14.0)
    e2 = sb.tile([2 * N, 2 * W], BF16, tag="e2")
    nc.scalar.activation(e2[:], u[:], ACT.Exp, bias=bv1[:], scale=-28.0)
    e4 = sb.tile([2 * N, 2 * W], BF16, tag="e4")
    nc.scalar.activation(e4[:], u[:], ACT.Exp, bias=bv2[:], scale=-56.0)

    # one PSUM bank (512 f32) per matmul group: HW rejects two accumulation
    # groups in one bank (CoreSim does not model this). 6 banks, single-buffered.
    BK = 512
    ps = pp.tile([128, 6 * BK], F32, tag="ps", bufs=1)
    for lv, e in enumerate((e1, e2, e4)):
        for b in range(B_LOC):
            nc.tensor.matmul(
                ps[:, (2 * lv + b) * BK : (2 * lv + b) * BK + W],
                e[32 * b : 32 * (b + 1), 0:W],          # y-factors (lhsT)
                e[32 * b : 32 * (b + 1), W : 2 * W],    # x-factors (rhs)
                start=True,
                stop=True,
            )

    # L = fp16(ln(S + 1e-30)); [L14 | L28 | L56] blocks of 256
    L = sb.tile([128, 6 * W], F16, tag="L")
    psv = ps[:].rearrange("p (q c) -> p q c", q=6)[:, :, 0:W]
    nc.scalar.activation(L[:], psv, ACT.Ln, bias=b0c[:], scale=1.0)
    L1 = L[:, 0 : 2 * W]
    L2 = L[:, 2 * W : 4 * W]
    L4 = L[:, 4 * W : 6 * W]

    # min-cascade in 64*log domain (all fp16, 2x/4x DVE modes)
    a2 = sb.tile([128, 2 * W], F16, tag="a2")
    nc.vector.tensor_scalar(a2[:], L2, 2.0, -K58, ALU.mult, ALU.add)
    dd = sb.tile([128, 2 * W], F16, tag="dd")
    nc.vector.tensor_tensor(dd[:], a2[:], L4, ALU.subtract)
    dc = sb.tile([128, 2 * W], F16, tag="dc")
    nc.vector.tensor_scalar(dc[:], dd[:], 0.0, LN2, ALU.max, ALU.min)
    h = sb.tile([128, 2 * W], F16, tag="h")
    nc.vector.tensor_scalar(h[:], dc[:], PC2, PC1, ALU.mult, ALU.add)
    g = sb.tile([128, 2 * W], F16, tag="g")
    nc.vector.tensor_tensor(g[:], h[:], dc[:], ALU.mult)
    a4 = sb.tile([128, 2 * W], F16, tag="a4")
    nc.vector.tensor_tensor(a4[:], L4, g[:], ALU.add)
    c14 = sb.tile([128, 2 * W], F16, tag="c14")
    nc.vector.tensor_scalar(c14[:], L1, 4.0, -3.0 * K58, ALU.mult, ALU.add)
    m1 = sb.tile([128, 2 * W], F16, tag="m1")
    nc.vector.tensor_tensor(m1[:], a4[:], a2[:], ALU.min)
    marg = sb.tile([128, 2 * W], F16, tag="marg")
    nc.vector.tensor_tensor(marg[:], m1[:], c14[:], ALU.min)

    res = sb.tile([128, 2 * W], F32, tag="res")
    nc.scalar.activation(
        res[:], marg[:], ACT.Exp, bias=ebc[:], scale=1.0 / 56.0
    )
    nc.sync.dma_start(
        att.ap().rearrange("b y x -> y b x"),
        res[:].rearrange("p (b x) -> p b x", b=B_LOC),
    )


def _get_nc():
    if "nc" not in _CACHE:
        _CACHE["nc"] = build_nc()
    return _CACHE["nc"]


def kernel(feature_map: np.ndarray, bboxes: np.ndarray) -> np.ndarray:
    nc = _get_nc()
    bb = np.ascontiguousarray(bboxes, dtype=np.float32)
    in_maps = [
        {"bb": bb[c * B_LOC : (c + 1) * B_LOC]} for c in range(N_CORES)
    ]
    res = run_bass_kernel_spmd(nc, in_maps, list(range(N_CORES)))
    out = np.concatenate([res.results[c]["att"] for c in range(N_CORES)], axis=0)
    return out[:, None, :, :].astype(np.float32, copy=False)
